# revision 1
# baseline (speedup 1.0000x reference)
"""Multi-head attention (B=16, GS=1024, E=768, H=12, D=64) on 8 trn2 NeuronCores.

Sharding: data-parallel over batch — 2 batches per core, no collectives.

Per-core design (per batch of S=1024 tokens):
  1. x^T via PE transpose:  xT [E, S] (bf16)
  2. qkT = (x @ w_qk)^T -> [2E, S] (head-dim on partitions)
     v   = x @ w_v -> [S, E] natural + a ones column per head
  3. heads processed in pairs (two 64-dim heads share the 128 PE rows via
     tile_position row groups): scoresT[ki,qi] matmuls, exp on ACT
     (scale=1/8 fused, no max subtraction — scores ~ N(0,1)), PV matmul
     with M=D+1 (ones column of v -> softmax denominator row for free).
     Denominators for all heads are gathered into one [H,S] tile (DMA from
     PSUM), reciprocal'd once per batch, broadcast via a DRAM bounce, and
     applied in-place to the attention output.
  4. proj: y = attnT^T @ w_proj + b_proj
Emission interleaves batch b+1's transpose/qkv/v work into batch b's
ACT-bound attention phase to keep the PE busy (and HAM-warm).
"""

import numpy as np
from contextlib import ExitStack

import concourse.bass as bass
import concourse.mybir as mybir
import concourse.tile as tile
from concourse import bacc

F32 = mybir.dt.float32
BF16 = mybir.dt.bfloat16
AF = mybir.ActivationFunctionType
P = 128


def build_nc(BPC=2, S=1024, E=768, H=12, D=64, act_dtype=BF16):
    SCALE = D ** -0.5
    E3 = 3 * E
    EC = E // P              # emb chunks
    SC = S // P              # seq chunks per batch
    QT = min(512, S)         # qi tile size
    NQT = S // QT            # qi tiles per batch
    HPC = P // D             # heads per 128-chunk (pair size)
    NPAIR = H // HPC
    T = BPC * S
    DV = D + 1               # v columns incl. ones
    NPLANE = (H + 3) // 4    # denominator tile planes (head -> partition 32*(h%4))

    nc = bacc.Bacc("TRN2", target_bir_lowering=False, debug=False)

    x_d = nc.dram_tensor("x_local", [E, T], act_dtype, kind="ExternalInput")
    wqkv_d = nc.dram_tensor("w_qkv", [E, E3], act_dtype, kind="ExternalInput")
    bqkv_d = nc.dram_tensor("b_qkv", [E3], F32, kind="ExternalInput")
    wproj_d = nc.dram_tensor("w_proj", [E, E], act_dtype, kind="ExternalInput")
    bproj_d = nc.dram_tensor("b_proj", [E], F32, kind="ExternalInput")
    y_d = nc.dram_tensor("y_local", [T, E], F32, kind="ExternalOutput")

    def bcast_part(ap, n):
        return bass.AP(tensor=ap.tensor, offset=ap.offset, ap=[[0, n]] + list(ap.ap))

    with tile.TileContext(nc) as tc, ExitStack() as ctx:
        const = ctx.enter_context(tc.tile_pool(name="const", bufs=1))
        xtp = ctx.enter_context(tc.tile_pool(name="xtp", bufs=1))
        qkp = ctx.enter_context(tc.tile_pool(name="qkp", bufs=2))
        vp = ctx.enter_context(tc.tile_pool(name="vp", bufs=2))
        atp = ctx.enter_context(tc.tile_pool(name="atp", bufs=2))
        expp = ctx.enter_context(tc.tile_pool(name="expp", bufs=2))
        outp = ctx.enter_context(tc.tile_pool(name="outp", bufs=2))
        denp = ctx.enter_context(tc.tile_pool(name="denp", bufs=1))
        rbp = ctx.enter_context(tc.tile_pool(name="rbp", bufs=2))
        ps_sc = ctx.enter_context(tc.tile_pool(name="ps_sc", bufs=2, space="PSUM"))
        ps_pv = ctx.enter_context(tc.tile_pool(name="ps_pv", bufs=2, space="PSUM"))
        ps_pr = ctx.enter_context(tc.tile_pool(name="ps_pr", bufs=2, space="PSUM"))
        dramp = ctx.enter_context(tc.tile_pool(name="dramp", bufs=2, space="DRAM"))

        # ---- weights (gpsimd DMA casts fp32 -> act_dtype); wqkv first ----
        wqkv_sb = const.tile([P, EC, E3], act_dtype, name="wqkv_sb")
        wproj_sb = const.tile([P, EC, E], act_dtype, name="wproj_sb")

        def emit_wqkv_loads():
            for ec in range(EC):
                nc.sync.dma_start(wqkv_sb[:, ec, :], wqkv_d[ec * P:(ec + 1) * P, :])

        warm = const.tile([P, 1], F32)
        nc.vector.memset(warm, 0.0)
        nc.scalar.activation(warm, warm, AF.Exp, scale=1.0)

        bqk_sb = const.tile([P, 2 * EC], F32)
        with nc.allow_non_contiguous_dma(reason="tiny strided bias load"):
            nc.sync.dma_start(bqk_sb, bqkv_d.ap()[0:2 * E].rearrange("(c p) -> p c", p=P))
        bv_bc = const.tile([P, E], act_dtype)
        nc.gpsimd.dma_start(bv_bc, bcast_part(bqkv_d.ap()[2 * E:3 * E], P))
        bproj_bc = const.tile([P, E], act_dtype)
        nc.gpsimd.dma_start(bproj_bc, bcast_part(bproj_d.ap(), P))
        for ec in range(EC):
            nc.sync.dma_start(wproj_sb[:, ec, :], wproj_d[ec * P:(ec + 1) * P, :])

        states = {}

        def make_prep_units(b, split_qi=False):
            st = states.setdefault(b, {})
            units = []

            def u_alloc():
                st["xT"] = [xtp.tile([P, EC, QT], act_dtype, name=f"xT{b}_{qi}",
                                     tag=f"xT{qi}") for qi in range(NQT)]
                st["qkT"] = qkp.tile([P, 2 * EC, S], act_dtype, name=f"qkT{b}", tag="qkT")
                st["v"] = vp.tile([P, SC, H, DV], act_dtype, name=f"v{b}", tag="v")
                st["attnT"] = atp.tile([P, EC, S], act_dtype, name=f"attnT{b}", tag="attnT")
                st["den"] = denp.tile([P, NPLANE, S], act_dtype, name=f"den{b}", tag="den")
                nc.vector.memset(st["v"][:, :, :, D:DV], 1.0)
                nc.gpsimd.memset(st["den"], 1.0)
            units.append(u_alloc)

            xt_view = x_d.ap().rearrange("(ec p) t -> p ec t", p=P)
            for qi in range(NQT):
                def u_xtr(qi=qi):
                    nc.sync.dma_start(
                        st["xT"][qi][:, :, :],
                        xt_view[:, :, b * S + qi * QT: b * S + (qi + 1) * QT])
                units.append(u_xtr)

            for m in range(2 * EC):
                if split_qi:
                    for qi in range(NQT):
                        def u_qk1(m=m, qi=qi):
                            pt = ps_pr.tile([P, 512], F32, tag="pr", name=f"qs{m}_{qi}")
                            for ec in range(EC):
                                nc.tensor.matmul(
                                    pt[:, 0:QT],
                                    wqkv_sb[:, ec, m * P:(m + 1) * P],
                                    st["xT"][qi][:, ec, :],
                                    start=(ec == 0), stop=(ec == EC - 1),
                                )
                            nc.vector.tensor_scalar_add(
                                st["qkT"][:, m, qi * QT:(qi + 1) * QT], pt[:, 0:QT],
                                bqk_sb[:, m:m + 1])
                        units.append(u_qk1)
                else:
                    def u_qk(m=m):
                        pts = [ps_pr.tile([P, 512], F32, tag="pr", name=f"qk{m}_{qi}")
                               for qi in range(NQT)]
                        for ec in range(EC):
                            for qi in range(NQT):
                                nc.tensor.matmul(
                                    pts[qi][:, 0:QT],
                                    wqkv_sb[:, ec, m * P:(m + 1) * P],
                                    st["xT"][qi][:, ec, :],
                                    start=(ec == 0), stop=(ec == EC - 1),
                                )
                        for qi in range(NQT):
                            nc.vector.tensor_scalar_add(
                                st["qkT"][:, m, qi * QT:(qi + 1) * QT], pts[qi][:, 0:QT],
                                bqk_sb[:, m:m + 1])
                    units.append(u_qk)

            nts = []
            nt0 = 0
            while nt0 < E:
                nts.append((nt0, min(512, E - nt0)))
                nt0 += min(512, E - nt0)
            for si in range(SC):
                def u_v(si=si):
                    pts = [ps_pr.tile([P, 512], F32, tag="pr", name=f"v{si}_{k}")
                           for k in range(len(nts))]
                    qi, so = divmod(si * P, QT)
                    for ec in range(EC):
                        for k, (nt, n_sl) in enumerate(nts):
                            nc.tensor.matmul(
                                pts[k][:, 0:n_sl],
                                st["xT"][qi][:, ec, so:so + P],
                                wqkv_sb[:, ec, 2 * E + nt: 2 * E + nt + n_sl],
                                start=(ec == 0), stop=(ec == EC - 1),
                            )
                    for k, (nt, n_sl) in enumerate(nts):
                        nh = n_sl // D
                        nc.vector.tensor_add(
                            st["v"][:, si, nt // D: nt // D + nh, 0:D],
                            pts[k][:, 0:n_sl].rearrange("p (h d) -> p h d", d=D),
                            bv_bc[:, nt:nt + n_sl].rearrange("p (h d) -> p h d", d=D))
                units.append(u_v)
            return units

        def make_head_units(b):
            st = states[b]
            units = []
            for pr in range(NPAIR):
                def u_pair(pr=pr):
                    qkT, v, attnT, den = st["qkT"], st["v"], st["attnT"], st["den"]
                    for qi in range(NQT):
                        ep = expp.tile([P, SC, HPC, QT], act_dtype, tag="exp")
                        for kc in range(SC):
                            ps = ps_sc.tile([P, HPC, 512], F32, tag="sc")
                            for j in range(HPC):
                                po = D * j
                                nc.tensor.matmul(
                                    ps[:, j, 0:QT],
                                    qkT[po:po + D, EC + pr, kc * P:(kc + 1) * P],
                                    qkT[po:po + D, pr, qi * QT:(qi + 1) * QT],
                                    start=True, stop=True,
                                    tile_position=(po, 0),
                                )
                            nc.scalar.activation(
                                ep[:, kc, :, :], ps[:, :, 0:QT], AF.Exp, scale=SCALE)
                        for j in range(HPC):
                            h = pr * HPC + j
                            po = D * j
                            pv = ps_pv.tile([P, 512], F32, tag="pv")
                            for kc in range(SC):
                                nc.tensor.matmul(
                                    pv[0:DV, 0:QT],
                                    v[:, kc, h, :],
                                    ep[:, kc, j, :],
                                    start=(kc == 0), stop=(kc == SC - 1),
                                )
                            dr = 32 * (h % 4)
                            nc.vector.tensor_copy(
                                den[dr:dr + 1, h // 4, qi * QT:(qi + 1) * QT],
                                pv[D:DV, 0:QT])
                            nc.vector.tensor_copy(
                                attnT[po:po + D, pr, qi * QT:(qi + 1) * QT],
                                pv[0:D, 0:QT])
                units.append(u_pair)
            return units

        def make_norm_units(b, pr_lo, pr_hi):
            st = states[b]

            def u_norm():
                attnT, den = st["attnT"], st["den"]
                h_lo, h_hi = pr_lo * HPC, pr_hi * HPC
                nh = h_hi - h_lo
                den_dense = denp.tile([H, S], act_dtype, tag="dend_sb",
                                      name=f"dd{b}_{pr_lo}")
                for k, h in enumerate(range(h_lo, h_hi)):
                    nc.sync.dma_start(den_dense[k:k + 1, :],
                                      den[32 * (h % 4):32 * (h % 4) + 1, h // 4, :])
                with nc.allow_low_precision(reason="softmax denom in act dtype"):
                    nc.vector.reciprocal(den_dense[0:nh, :], den_dense[0:nh, :])
                den_dr = dramp.tile([H, S], act_dtype, tag="dend", name=f"dr{b}_{pr_lo}")
                nc.sync.dma_start(den_dr[0:nh, :], den_dense[0:nh, :])
                for pr in range(pr_lo, pr_hi):
                    rb = rbp.tile([P, S], act_dtype, tag="rb")
                    for j in range(HPC):
                        k = (pr - pr_lo) * HPC + j
                        nc.sync.dma_start(rb[D * j:D * (j + 1), :],
                                          bcast_part(den_dr[k, :], D))
                    nc.vector.tensor_mul(attnT[:, pr, :], attnT[:, pr, :], rb)
            return u_norm

        def make_norm_proj_units(b, pr_split=0):
            st = states[b]
            units = []
            if pr_split:
                units.append(make_norm_units(b, pr_split, NPAIR))
            else:
                units.append(make_norm_units(b, 0, NPAIR))

            nts = []
            nt0 = 0
            while nt0 < E:
                nts.append((nt0, min(512, E - nt0)))
                nt0 += min(512, E - nt0)
            for si in range(SC):
                def u_proj(si=si):
                    attnT = st["attnT"]
                    yt = outp.tile([P, E], F32, tag="y")
                    pts = [ps_pr.tile([P, 512], F32, tag="pr", name=f"pj{si}_{k}")
                           for k in range(len(nts))]
                    for ec in range(EC):
                        for k, (nt, n_sl) in enumerate(nts):
                            nc.tensor.matmul(
                                pts[k][:, 0:n_sl],
                                attnT[:, ec, si * P:(si + 1) * P],
                                wproj_sb[:, ec, nt:nt + n_sl],
                                start=(ec == 0), stop=(ec == EC - 1),
                            )
                    for k, (nt, n_sl) in enumerate(nts):
                        nc.vector.tensor_add(yt[:, nt:nt + n_sl], pts[k][:, 0:n_sl],
                                             bproj_bc[:, nt:nt + n_sl])
                    nc.sync.dma_start(y_d[b * S + si * P: b * S + (si + 1) * P, :], yt)
                units.append(u_proj)
            return units

        # ---------- emission schedule ----------
        prep0 = make_prep_units(0, split_qi=True)
        prep0[0]()          # tile allocs
        prep0[1]()          # xT(b0, qi0) DMA ahead of the weight stream
        emit_wqkv_loads()
        for u in prep0[2:]:
            u()
        carry = []
        for b in range(BPC):
            head_units = make_head_units(b)
            filler = list(carry)
            if b + 1 < BPC:
                filler += make_prep_units(b + 1)
            last = b == BPC - 1
            pr_split = max(1, NPAIR - 1) if last else 0
            carry = make_norm_proj_units(b, pr_split=pr_split)
            early_norm = make_norm_units(b, 0, pr_split) if last else None
            nslot = len(head_units)
            slots = [[] for _ in range(nslot)]
            for i, fu in enumerate(filler):
                slots[min(i * nslot // max(len(filler), 1), nslot - 1)].append(fu)
            for p, hu in enumerate(head_units):
                hu()
                if early_norm is not None and p == pr_split - 1:
                    slots[p].append(early_norm)
                for fu in slots[p]:
                    fu()
        for u in carry:
            u()

    nc.compile()
    return nc


_NC_CACHE = {}


def _get_nc():
    if "nc" not in _NC_CACHE:
        _NC_CACHE["nc"] = build_nc()
    return _NC_CACHE["nc"]


B, GS, E_FULL = 16, 1024, 768
N_CORES = 8
BPC_FULL = B // N_CORES


def make_in_maps(x, w_qkv, b_qkv, w_proj, b_proj):
    import ml_dtypes
    bf = ml_dtypes.bfloat16
    x = np.asarray(x, dtype=np.float32).astype(bf)  # [B, GS, E]
    w_qkv = np.ascontiguousarray(np.asarray(w_qkv, dtype=np.float32).astype(bf))
    b_qkv = np.ascontiguousarray(np.asarray(b_qkv, dtype=np.float32))
    w_proj = np.ascontiguousarray(np.asarray(w_proj, dtype=np.float32).astype(bf))
    b_proj = np.ascontiguousarray(np.asarray(b_proj, dtype=np.float32))
    in_maps = []
    for i in range(N_CORES):
        in_maps.append({
            "x_local": np.ascontiguousarray(
                x[i * BPC_FULL:(i + 1) * BPC_FULL].reshape(BPC_FULL * GS, E_FULL).T),
            "w_qkv": w_qkv, "b_qkv": b_qkv,
            "w_proj": w_proj, "b_proj": b_proj,
        })
    return in_maps


def gather_out(results):
    return np.concatenate(
        [r["y_local"].reshape(BPC_FULL, GS, E_FULL) for r in results],
        axis=0).astype(np.float32)


def kernel(x, w_qkv, b_qkv, w_proj, b_proj):
    from concourse.bass_utils import run_bass_kernel_spmd

    nc = _get_nc()
    in_maps = make_in_maps(x, w_qkv, b_qkv, w_proj, b_proj)
    res = run_bass_kernel_spmd(nc, in_maps, core_ids=list(range(N_CORES)))
    return gather_out(res.results)



# revision 15
# speedup vs baseline: 1.0656x; 1.0656x over previous
"""Multi-head attention (B=16, GS=1024, E=768, H=12, D=64) on 8 trn2 NeuronCores.

Sharding: data-parallel over batch — 2 batches per core, no collectives.

Per-core design (per batch of S=1024 tokens):
  1. x arrives pre-transposed in DRAM as [E, T]; DMA'd to SBUF xT tiles.
  2. qkT = (x @ w_qk)^T -> [2E, S] (head-dim on partitions), emitted as
     per-(m-chunk, qi) "atoms" of 6 accumulating matmuls + DVE evac/bias.
     v = x @ w_v -> [S, E] natural + a ones column per head.
  3. heads processed in pairs (two 64-dim heads share the 128 PE rows via
     tile_position row groups). Per (pair, qi) the kc loop is emitted
     fine-grained: scoresT MMs (both heads, concurrent row groups) ->
     exp on ACT (scale=1/8 fused, no max subtraction — scores ~ N(0,1))
     -> PV MMs (M=D+1: ones column of v gives the softmax denominator row
     for free). Prep/proj filler atoms are injected between kc steps so
     the PE stays busy while ACT streams exp.
  4. denominators: gathered per pair-group into [H,S] f32, inverted with
     reciprocal_approx_fast, broadcast via a DRAM bounce, applied in-place.
  5. proj: y = attnT^T @ w_proj + b_proj, bf16 out (host casts to f32).
Weight DMA is split in halves and ordered so pair 0's scores can start
~4us in; k-side m-chunks are emitted before q-side so the scores
stationary (full-S k) is ready first; v atoms early so PV never starves.
"""

import numpy as np
from collections import deque
from contextlib import ExitStack

import concourse.bass as bass
import concourse.mybir as mybir
import concourse.tile as tile
from concourse import bacc

F32 = mybir.dt.float32
BF16 = mybir.dt.bfloat16
AF = mybir.ActivationFunctionType
P = 128
FOLLOW = False


def build_nc(BPC=2, S=1024, E=768, H=12, D=64, act_dtype=BF16, debug_dump=False):
    SCALE = D ** -0.5
    E3 = 3 * E
    EC = E // P              # emb chunks
    SC = S // P              # seq chunks per batch
    QT = min(512, S)         # qi tile size
    NQT = S // QT            # qi tiles per batch
    HPC = P // D             # heads per 128-chunk (pair size)
    NPAIR = H // HPC
    T = BPC * S
    DV = D + 1               # v columns incl. ones
    NPLANE = (H + 3) // 4    # denominator tile planes (head -> partition 32*(h%4))

    nc = bacc.Bacc("TRN2", target_bir_lowering=False, debug=False)

    x_d = nc.dram_tensor("x_local", [E, T], act_dtype, kind="ExternalInput")
    wqkv_d = nc.dram_tensor("w_qkv", [E, E3], act_dtype, kind="ExternalInput")
    bqkv_d = nc.dram_tensor("b_qkv", [E3], F32, kind="ExternalInput")
    wproj_d = nc.dram_tensor("w_proj", [E, E], act_dtype, kind="ExternalInput")
    bproj_d = nc.dram_tensor("b_proj", [E], F32, kind="ExternalInput")
    y_d = nc.dram_tensor("y_local", [T, E], act_dtype, kind="ExternalOutput")

    def bcast_part(ap, n):
        return bass.AP(tensor=ap.tensor, offset=ap.offset, ap=[[0, n]] + list(ap.ap))

    with tile.TileContext(nc) as tc, ExitStack() as ctx:
        const = ctx.enter_context(tc.tile_pool(name="const", bufs=1))
        xtp = ctx.enter_context(tc.tile_pool(name="xtp", bufs=1))
        qkp = ctx.enter_context(tc.tile_pool(name="qkp", bufs=2))
        vp = ctx.enter_context(tc.tile_pool(name="vp", bufs=2))
        atp = ctx.enter_context(tc.tile_pool(name="atp", bufs=2))
        expp = ctx.enter_context(tc.tile_pool(name="expp", bufs=2))
        outp = ctx.enter_context(tc.tile_pool(name="outp", bufs=2))
        denp = ctx.enter_context(tc.tile_pool(name="denp", bufs=1))
        rbp = ctx.enter_context(tc.tile_pool(name="rbp", bufs=2))
        ps_sc = ctx.enter_context(tc.tile_pool(name="ps_sc", bufs=2, space="PSUM"))
        ps_pv = ctx.enter_context(tc.tile_pool(name="ps_pv", bufs=2, space="PSUM"))
        ps_pr = ctx.enter_context(tc.tile_pool(name="ps_pr", bufs=2, space="PSUM"))
        dramp = ctx.enter_context(tc.tile_pool(name="dramp", bufs=2, space="DRAM"))

        wqkv_sb = const.tile([P, EC, E3], act_dtype, name="wqkv_sb")
        wproj_sb = const.tile([P, EC, E], act_dtype, name="wproj_sb")
        WH = E3 // 2           # wqkv DMA half width

        warm = const.tile([P, 1], F32)
        nc.vector.memset(warm, 0.0)
        nc.scalar.activation(warm, warm, AF.Exp, scale=1.0)

        bqk_sb = const.tile([P, 2 * EC], F32)
        with nc.allow_non_contiguous_dma(reason="tiny strided bias load"):
            nc.sync.dma_start(bqk_sb, bqkv_d.ap()[0:2 * E].rearrange("(c p) -> p c", p=P))
        bv_bc = const.tile([P, E], act_dtype)
        nc.gpsimd.dma_start(bv_bc, bcast_part(bqkv_d.ap()[2 * E:3 * E], P))
        bproj_bc = const.tile([P, E], act_dtype)
        nc.gpsimd.dma_start(bproj_bc, bcast_part(bproj_d.ap(), P))

        def emit_w_half(half):
            for ec in range(EC):
                nc.sync.dma_start(wqkv_sb[:, ec, half * WH:(half + 1) * WH],
                                  wqkv_d[ec * P:(ec + 1) * P, half * WH:(half + 1) * WH])

        def emit_wproj():
            for ec in range(EC):
                nc.sync.dma_start(wproj_sb[:, ec, :], wproj_d[ec * P:(ec + 1) * P, :])

        states = {}
        xt_view = x_d.ap().rearrange("(ec p) t -> p ec t", p=P)

        # order m-chunks so pair p's k (EC+p) then q (p) arrive first
        M_ORDER = []
        for pr in range(NPAIR):
            M_ORDER += [EC + pr, pr]

        nts = []
        nt0 = 0
        while nt0 < E:
            nts.append((nt0, min(512, E - nt0)))
            nt0 += min(512, E - nt0)

        # v tiles for all batches allocated and ones-initialized up front:
        # nothing ever writes col D again, so the ones survive both batches
        # (a deferred per-batch memset raced with PV's stationary reads).
        for b in range(BPC):
            st = states.setdefault(b, {})
            st["v"] = vp.tile([P, SC, H, DV], act_dtype, name=f"v{b}", tag="v")
            nc.vector.memset(st["v"][:, :, :, D:DV], 1.0)

        def make_alloc_unit(b):
            st = states[b]

            def u_alloc():
                st["xT"] = [xtp.tile([P, EC, QT], act_dtype, name=f"xT{b}_{qi}",
                                     tag=f"xT{qi}") for qi in range(NQT)]
                st["qkT"] = qkp.tile([P, 2 * EC, S], act_dtype, name=f"qkT{b}", tag="qkT")
                st["attnT"] = atp.tile([P, EC, S], act_dtype, name=f"attnT{b}", tag="attnT")
                st["den"] = denp.tile([P, NPLANE, S], F32, name=f"den{b}", tag="den")
            return u_alloc

        def make_xt_unit(b, qi):
            def u_xtr():
                nc.sync.dma_start(
                    states[b]["xT"][qi][:, :, :],
                    xt_view[:, :, b * S + qi * QT: b * S + (qi + 1) * QT])
            return u_xtr

        def make_qk_atom(b, m, qi):
            def u_qk():
                st = states[b]
                pt = ps_pr.tile([P, 512], F32, tag="pr", name=f"qk{b}_{m}_{qi}")
                for ec in range(EC):
                    nc.tensor.matmul(
                        pt[:, 0:QT],
                        wqkv_sb[:, ec, m * P:(m + 1) * P],
                        st["xT"][qi][:, ec, :],
                        start=(ec == 0), stop=(ec == EC - 1),
                    )
                ev = nc.vector.tensor_scalar_add(
                    st["qkT"][:, m, qi * QT:(qi + 1) * QT], pt[:, 0:QT],
                    bqk_sb[:, m:m + 1])
                st.setdefault("qk_evac", {})[(m, qi)] = ev
                if FOLLOW and b == 1 and m in (0, EC):
                    tile.tile_follow(ev)
            return u_qk

        def make_v_atom(b, si, k):
            def u_v():
                st = states[b]
                nt, n_sl = nts[k]
                pt = ps_pr.tile([P, 512], F32, tag="pr", name=f"v{b}_{si}_{k}")
                qi, so = divmod(si * P, QT)
                for ec in range(EC):
                    nc.tensor.matmul(
                        pt[:, 0:n_sl],
                        st["xT"][qi][:, ec, so:so + P],
                        wqkv_sb[:, ec, 2 * E + nt: 2 * E + nt + n_sl],
                        start=(ec == 0), stop=(ec == EC - 1),
                    )
                nh = n_sl // D
                nc.vector.tensor_add(
                    st["v"][:, si, nt // D: nt // D + nh, 0:D],
                    pt[:, 0:n_sl].rearrange("p (h d) -> p h d", d=D),
                    bv_bc[:, nt:nt + n_sl].rearrange("p (h d) -> p h d", d=D))
            return u_v

        def make_proj_atom(b, si, k):
            def u_proj():
                st = states[b]
                nt, n_sl = nts[k]
                yt = st["yt"][si]
                pt = ps_pr.tile([P, 512], F32, tag="pr", name=f"pj{b}_{si}_{k}")
                for ec in range(EC):
                    nc.tensor.matmul(
                        pt[:, 0:n_sl],
                        st["attnT"][:, ec, si * P:(si + 1) * P],
                        wproj_sb[:, ec, nt:nt + n_sl],
                        start=(ec == 0), stop=(ec == EC - 1),
                    )
                nc.vector.tensor_add(yt[:, nt:nt + n_sl], pt[:, 0:n_sl],
                                     bproj_bc[:, nt:nt + n_sl])
                if k == len(nts) - 1:
                    nc.sync.dma_start(
                        y_d[b * S + si * P: b * S + (si + 1) * P, :], yt)
            return u_proj

        def make_proj_alloc(b):
            def u():
                states[b]["yt"] = [outp.tile([P, E], act_dtype, tag=f"y{si % 2}",
                                             name=f"yt{b}_{si}") for si in range(SC)]
            return u

        done = set()

        def pull_one(filler):
            key, fn = filler.popleft()
            fn()
            if key is not None:
                done.add(key)

        def ensure(filler, keys):
            """Force-pull filler atoms (in order) until all keys are emitted —
            a consumer must never be emitted before its producers."""
            for key in keys:
                while key not in done:
                    pull_one(filler)

        def run_head_pair(b, pr, qi, filler):
            """Emit one (pair, qi) attention stream: per-kc scores -> exp -> PV,
            pulling one filler atom every other kc step."""
            ensure(filler, [("alloc", b), ("xt", b, 0), ("xt", b, 1),
                            ("qk", b, pr, qi)])
            st = states[b]
            qkT, v, attnT, den = st["qkT"], st["v"], st["attnT"], st["den"]
            vnt = 0 if 2 * pr * D < nts[0][1] else 1
            ep = expp.tile([P, SC, HPC, QT], act_dtype, tag="exp")
            pvs = [ps_pv.tile([P, 512], F32, tag="pv", name=f"pv{b}_{pr}_{qi}_{j}")
                   for j in range(HPC)]
            for kc in range(SC):
                ensure(filler, [("qk", b, EC + pr, kc * P // QT),
                                ("v", b, kc, vnt)])
                ps = ps_sc.tile([P, HPC, 512], F32, tag="sc")
                for j in range(HPC):
                    po = D * j
                    mm = nc.tensor.matmul(
                        ps[:, j, 0:QT],
                        qkT[po:po + D, EC + pr, kc * P:(kc + 1) * P],
                        qkT[po:po + D, pr, qi * QT:(qi + 1) * QT],
                        start=True, stop=True,
                        tile_position=(po, 0),
                    )
                    if FOLLOW and b == 1 and pr == 0 and qi == 0 and kc == 0:
                        tile.tile_follow(mm, log_all_deps=True)
                nc.scalar.activation(
                    ep[:, kc, :, :], ps[:, :, 0:QT], AF.Exp, scale=SCALE)
                for j in range(HPC):
                    h = pr * HPC + j
                    nc.tensor.matmul(
                        pvs[j][0:DV, 0:QT],
                        v[:, kc, h, :],
                        ep[:, kc, j, :],
                        start=(kc == 0), stop=(kc == SC - 1),
                    )
                if kc % 2 == 1 and filler:
                    pull_one(filler)
            for j in range(HPC):
                h = pr * HPC + j
                po = D * j
                dr = 32 * (h % 4)
                nc.vector.tensor_copy(
                    den[dr:dr + 1, h // 4, qi * QT:(qi + 1) * QT],
                    pvs[j][D:DV, 0:QT])
                nc.vector.tensor_copy(
                    attnT[po:po + D, pr, qi * QT:(qi + 1) * QT],
                    pvs[j][0:D, 0:QT])

        def make_norm_units(b, pr_lo, pr_hi):
            """Returns (u_prep, u_mul): prep gathers denominators, inverts
            them and launches the broadcast DMAs; mul applies them."""
            rbs = {}

            def u_prep():
                st = states[b]
                den = st["den"]
                h_lo, h_hi = pr_lo * HPC, pr_hi * HPC
                nh = h_hi - h_lo
                den_dense = denp.tile([H, S], F32, tag="dend_sb",
                                      name=f"dd{b}_{pr_lo}")
                for k, h in enumerate(range(h_lo, h_hi)):
                    nc.gpsimd.dma_start(den_dense[k:k + 1, :],
                                        den[32 * (h % 4):32 * (h % 4) + 1, h // 4, :])
                nc.vector.reciprocal_approx_fast(den_dense[0:nh, :],
                                                 den_dense[0:nh, :])
                den_dr = dramp.tile([H, S], act_dtype, tag="dend", name=f"dr{b}_{pr_lo}")
                nc.gpsimd.dma_start(den_dr[0:nh, :], den_dense[0:nh, :])
                for pr in range(pr_lo, pr_hi):
                    rb = rbp.tile([P, S], act_dtype, tag="rb")
                    for j in range(HPC):
                        k = (pr - pr_lo) * HPC + j
                        nc.gpsimd.dma_start(rb[D * j:D * (j + 1), :],
                                            bcast_part(den_dr[k, :], D))
                    rbs[pr] = rb

            def u_mul():
                attnT = states[b]["attnT"]
                for pr in range(pr_lo, pr_hi):
                    nc.vector.tensor_mul(attnT[:, pr, :], attnT[:, pr, :], rbs[pr])
            return u_prep, u_mul

        # ---------- emission schedule ----------
        # startup: batch-0 alloc, xT DMAs, first weight half, then prime pair 0
        make_alloc_unit(0)()
        done.add(("alloc", 0))
        make_xt_unit(0, 0)()
        done.add(("xt", 0, 0))
        emit_w_half(0)
        make_xt_unit(0, 1)()
        done.add(("xt", 0, 1))
        for m in M_ORDER[:2]:               # k(pair0), then q(pair0)
            for qi in range(NQT):
                make_qk_atom(0, m, qi)()
                done.add(("qk", 0, m, qi))
        emit_w_half(1)
        emit_wproj()

        # leftover prep-0 atoms, prioritized: pair1 qk, all v(nt0), then rest
        prep0 = deque()
        for m in M_ORDER[2:4]:
            for qi in range(NQT):
                prep0.append((("qk", 0, m, qi), make_qk_atom(0, m, qi)))
        for si in range(SC):
            prep0.append((("v", 0, si, 0), make_v_atom(0, si, 0)))
        for m in M_ORDER[4:]:
            for qi in range(NQT):
                prep0.append((("qk", 0, m, qi), make_qk_atom(0, m, qi)))
        for si in range(SC):
            prep0.append((("v", 0, si, 1), make_v_atom(0, si, 1)))

        filler = prep0
        norm_muls = {}
        for b in range(BPC):
            last = b == BPC - 1
            # build next batch's prep atoms / previous batch's proj atoms
            nxt = deque()
            if not last:
                bn = b + 1
                nxt.append((("alloc", bn), make_alloc_unit(bn)))
                nxt.append((("xt", bn, 0), make_xt_unit(bn, 0)))
                nxt.append((("xt", bn, 1), make_xt_unit(bn, 1)))
                for m in M_ORDER:
                    for qi in range(NQT):
                        nxt.append((("qk", bn, m, qi), make_qk_atom(bn, m, qi)))
                    if m == EC:          # after k(pair0), interleave v atoms
                        for si in range(SC):
                            nxt.append((("v", bn, si, 0), make_v_atom(bn, si, 0)))
                for si in range(SC):
                    nxt.append((("v", bn, si, 1), make_v_atom(bn, si, 1)))
            if b > 0:
                bp = b - 1
                nxt.append((None, norm_muls[bp]))
                nxt.append((None, make_proj_alloc(bp)))
                for si in range(SC):
                    for k in range(len(nts)):
                        nxt.append((None, make_proj_atom(bp, si, k)))
            filler.extend(nxt)

            tail_groups = []
            if last:
                # per-group norm for the last batch; tiny final group keeps
                # the critical den->recip->broadcast chain short
                groups = [(0, 3, 2), (3, 5, 4), (5, NPAIR, NPAIR - 1)]
            for pr in range(NPAIR):
                for qi in range(NQT):
                    run_head_pair(b, pr, qi, filler)
                if last:
                    for (lo, hi, after) in groups:
                        if pr == after:
                            if pr == NPAIR - 1:
                                while filler:
                                    pull_one(filler)
                            up, um = make_norm_units(b, lo, hi)
                            up()
                            tail_groups.append(um)
            if not last:
                up, um = make_norm_units(b, 0, NPAIR)
                up()
                norm_muls[b] = um

        # tail: apply last-batch norms, then proj
        for um in tail_groups:
            um()
        bl = BPC - 1
        make_proj_alloc(bl)()
        for si in range(SC):
            for k in range(len(nts)):
                make_proj_atom(bl, si, k)()

        if debug_dump:
            st = states[bl]
            dq = nc.dram_tensor("dbg_qkT", [P, 2 * EC, S], act_dtype,
                                kind="ExternalOutput")
            dv = nc.dram_tensor("dbg_v", [P, SC, H, DV], act_dtype,
                                kind="ExternalOutput")
            da = nc.dram_tensor("dbg_attnT", [P, EC, S], act_dtype,
                                kind="ExternalOutput")
            dd = nc.dram_tensor("dbg_den", [P, NPLANE, S], F32,
                                kind="ExternalOutput")
            nc.sync.dma_start(dq.ap(), st["qkT"][:, :, :])
            nc.sync.dma_start(dv.ap(), st["v"][:, :, :, :])
            nc.sync.dma_start(da.ap(), st["attnT"][:, :, :])
            nc.sync.dma_start(dd.ap(), st["den"][:, :, :])

    nc.compile()
    return nc


_NC_CACHE = {}


def _get_nc():
    if "nc" not in _NC_CACHE:
        _NC_CACHE["nc"] = build_nc()
    return _NC_CACHE["nc"]


B, GS, E_FULL = 16, 1024, 768
N_CORES = 8
BPC_FULL = B // N_CORES


def make_in_maps(x, w_qkv, b_qkv, w_proj, b_proj):
    import ml_dtypes
    bf = ml_dtypes.bfloat16
    x = np.asarray(x, dtype=np.float32).astype(bf)  # [B, GS, E]
    w_qkv = np.ascontiguousarray(np.asarray(w_qkv, dtype=np.float32).astype(bf))
    b_qkv = np.ascontiguousarray(np.asarray(b_qkv, dtype=np.float32))
    w_proj = np.ascontiguousarray(np.asarray(w_proj, dtype=np.float32).astype(bf))
    b_proj = np.ascontiguousarray(np.asarray(b_proj, dtype=np.float32))
    in_maps = []
    for i in range(N_CORES):
        in_maps.append({
            "x_local": np.ascontiguousarray(
                x[i * BPC_FULL:(i + 1) * BPC_FULL].reshape(BPC_FULL * GS, E_FULL).T),
            "w_qkv": w_qkv, "b_qkv": b_qkv,
            "w_proj": w_proj, "b_proj": b_proj,
        })
    return in_maps


def gather_out(results):
    return np.concatenate(
        [r["y_local"].reshape(BPC_FULL, GS, E_FULL) for r in results],
        axis=0).astype(np.float32)


def kernel(x, w_qkv, b_qkv, w_proj, b_proj):
    from concourse.bass_utils import run_bass_kernel_spmd

    nc = _get_nc()
    in_maps = make_in_maps(x, w_qkv, b_qkv, w_proj, b_proj)
    res = run_bass_kernel_spmd(nc, in_maps, core_ids=list(range(N_CORES)))
    return gather_out(res.results)


# revision 18
# speedup vs baseline: 1.0786x; 1.0122x over previous
"""Multi-head attention (B=16, GS=1024, E=768, H=12, D=64) on 8 trn2 NeuronCores.

Sharding: data-parallel over batch — 2 batches per core, no collectives.

Per-core design (per batch of S=1024 tokens):
  1. x arrives pre-transposed in DRAM as [E, T]; DMA'd to SBUF xT tiles.
  2. qkT = (x @ w_qk)^T -> [2E, S] (head-dim on partitions), emitted as
     per-(m-chunk, qi) "atoms" of 6 accumulating matmuls + DVE evac/bias.
     v = x @ w_v -> [S, E] natural + a ones column per head.
  3. heads processed in pairs (two 64-dim heads share the 128 PE rows via
     tile_position row groups). Per (pair, qi) the kc loop is emitted
     fine-grained: scoresT MMs (both heads, concurrent row groups) ->
     exp on ACT (scale=1/8 fused, no max subtraction — scores ~ N(0,1))
     -> PV MMs (M=D+1: ones column of v gives the softmax denominator row
     for free). Prep/proj filler atoms are injected between kc steps so
     the PE stays busy while ACT streams exp.
  4. denominators: gathered per pair-group into [H,S] f32, inverted with
     reciprocal_approx_fast, broadcast via a DRAM bounce, applied in-place.
  5. proj: y = attnT^T @ w_proj + b_proj, bf16 out (host casts to f32).
Weight DMA is split in halves and ordered so pair 0's scores can start
~4us in; k-side m-chunks are emitted before q-side so the scores
stationary (full-S k) is ready first; v atoms early so PV never starves.
"""

import numpy as np
from collections import deque
from contextlib import ExitStack

import concourse.bass as bass
import concourse.mybir as mybir
import concourse.tile as tile
from concourse import bacc

F32 = mybir.dt.float32
BF16 = mybir.dt.bfloat16
AF = mybir.ActivationFunctionType
P = 128
FOLLOW = False


def build_nc(BPC=2, S=1024, E=768, H=12, D=64, act_dtype=BF16, debug_dump=False):
    SCALE = D ** -0.5
    E3 = 3 * E
    EC = E // P              # emb chunks
    SC = S // P              # seq chunks per batch
    QT = min(512, S)         # qi tile size
    NQT = S // QT            # qi tiles per batch
    HPC = P // D             # heads per 128-chunk (pair size)
    NPAIR = H // HPC
    T = BPC * S
    DV = D + 1               # v columns incl. ones
    NPLANE = (H + 3) // 4    # denominator tile planes (head -> partition 32*(h%4))

    nc = bacc.Bacc("TRN2", target_bir_lowering=False, debug=False)

    x_d = nc.dram_tensor("x_local", [E, T], act_dtype, kind="ExternalInput")
    wqkv_d = nc.dram_tensor("w_qkv", [E, E3], act_dtype, kind="ExternalInput")
    bqkv_d = nc.dram_tensor("b_qkv", [E3], F32, kind="ExternalInput")
    wproj_d = nc.dram_tensor("w_proj", [E, E], act_dtype, kind="ExternalInput")
    bproj_d = nc.dram_tensor("b_proj", [E], F32, kind="ExternalInput")
    y_d = nc.dram_tensor("y_local", [T, E], act_dtype, kind="ExternalOutput")

    def bcast_part(ap, n):
        return bass.AP(tensor=ap.tensor, offset=ap.offset, ap=[[0, n]] + list(ap.ap))

    with tile.TileContext(nc) as tc, ExitStack() as ctx:
        const = ctx.enter_context(tc.tile_pool(name="const", bufs=1))
        xtp = ctx.enter_context(tc.tile_pool(name="xtp", bufs=1))
        qkp = ctx.enter_context(tc.tile_pool(name="qkp", bufs=2))
        vp = ctx.enter_context(tc.tile_pool(name="vp", bufs=2))
        atp = ctx.enter_context(tc.tile_pool(name="atp", bufs=2))
        expp = ctx.enter_context(tc.tile_pool(name="expp", bufs=2))
        outp = ctx.enter_context(tc.tile_pool(name="outp", bufs=2))
        denp = ctx.enter_context(tc.tile_pool(name="denp", bufs=1))
        rbp = ctx.enter_context(tc.tile_pool(name="rbp", bufs=2))
        ps_sc = ctx.enter_context(tc.tile_pool(name="ps_sc", bufs=2, space="PSUM"))
        ps_pv = ctx.enter_context(tc.tile_pool(name="ps_pv", bufs=2, space="PSUM"))
        ps_pr = ctx.enter_context(tc.tile_pool(name="ps_pr", bufs=2, space="PSUM"))
        dramp = ctx.enter_context(tc.tile_pool(name="dramp", bufs=2, space="DRAM"))

        wqkv_sb = const.tile([P, EC, E3], act_dtype, name="wqkv_sb")
        wproj_sb = const.tile([P, EC, E], act_dtype, name="wproj_sb")
        WH = E3 // 2           # wqkv DMA half width

        warm = const.tile([P, 1], F32)
        nc.vector.memset(warm, 0.0)
        nc.scalar.activation(warm, warm, AF.Exp, scale=1.0)

        bqk_sb = const.tile([P, 2 * EC], F32)
        with nc.allow_non_contiguous_dma(reason="tiny strided bias load"):
            nc.sync.dma_start(bqk_sb, bqkv_d.ap()[0:2 * E].rearrange("(c p) -> p c", p=P))
        bv_bc = const.tile([P, E], act_dtype)
        nc.gpsimd.dma_start(bv_bc, bcast_part(bqkv_d.ap()[2 * E:3 * E], P))
        bproj_bc = const.tile([P, E], act_dtype)
        nc.gpsimd.dma_start(bproj_bc, bcast_part(bproj_d.ap(), P))

        def emit_w_half(half):
            for ec in range(EC):
                nc.sync.dma_start(wqkv_sb[:, ec, half * WH:(half + 1) * WH],
                                  wqkv_d[ec * P:(ec + 1) * P, half * WH:(half + 1) * WH])

        def emit_wproj():
            for ec in range(EC):
                nc.sync.dma_start(wproj_sb[:, ec, :], wproj_d[ec * P:(ec + 1) * P, :])

        states = {}
        xt_view = x_d.ap().rearrange("(ec p) t -> p ec t", p=P)

        # order m-chunks so pair p's k (EC+p) then q (p) arrive first
        M_ORDER = []
        for pr in range(NPAIR):
            M_ORDER += [EC + pr, pr]

        nts = []
        nt0 = 0
        while nt0 < E:
            nts.append((nt0, min(512, E - nt0)))
            nt0 += min(512, E - nt0)

        # v tiles for all batches allocated and ones-initialized up front:
        # nothing ever writes col D again, so the ones survive both batches
        # (a deferred per-batch memset raced with PV's stationary reads).
        for b in range(BPC):
            st = states.setdefault(b, {})
            st["v"] = vp.tile([P, SC, H, DV], act_dtype, name=f"v{b}", tag="v")
            nc.vector.memset(st["v"][:, :, :, D:DV], 1.0)

        def make_alloc_unit(b):
            st = states[b]

            def u_alloc():
                st["xT"] = [xtp.tile([P, EC, QT], act_dtype, name=f"xT{b}_{qi}",
                                     tag=f"xT{qi}") for qi in range(NQT)]
                st["qkT"] = qkp.tile([P, 2 * EC, S], act_dtype, name=f"qkT{b}", tag="qkT")
                st["attnT"] = atp.tile([P, EC, S], act_dtype, name=f"attnT{b}", tag="attnT")
                st["den"] = denp.tile([P, NPLANE, S], F32, name=f"den{b}", tag="den")
            return u_alloc

        def make_xt_unit(b, qi):
            def u_xtr():
                nc.sync.dma_start(
                    states[b]["xT"][qi][:, :, :],
                    xt_view[:, :, b * S + qi * QT: b * S + (qi + 1) * QT])
            return u_xtr

        def make_qk_atom(b, m, qi):
            def u_qk():
                st = states[b]
                pt = ps_pr.tile([P, 512], F32, tag="pr", name=f"qk{b}_{m}_{qi}")
                for ec in range(EC):
                    nc.tensor.matmul(
                        pt[:, 0:QT],
                        wqkv_sb[:, ec, m * P:(m + 1) * P],
                        st["xT"][qi][:, ec, :],
                        start=(ec == 0), stop=(ec == EC - 1),
                    )
                ev = nc.vector.tensor_scalar_add(
                    st["qkT"][:, m, qi * QT:(qi + 1) * QT], pt[:, 0:QT],
                    bqk_sb[:, m:m + 1])
                st.setdefault("qk_evac", {})[(m, qi)] = ev
                if FOLLOW and b == 1 and m in (0, EC):
                    tile.tile_follow(ev)
            return u_qk

        def make_v_atom(b, si, k):
            def u_v():
                st = states[b]
                nt, n_sl = nts[k]
                pt = ps_pr.tile([P, 512], F32, tag="pr", name=f"v{b}_{si}_{k}")
                qi, so = divmod(si * P, QT)
                for ec in range(EC):
                    nc.tensor.matmul(
                        pt[:, 0:n_sl],
                        st["xT"][qi][:, ec, so:so + P],
                        wqkv_sb[:, ec, 2 * E + nt: 2 * E + nt + n_sl],
                        start=(ec == 0), stop=(ec == EC - 1),
                    )
                nh = n_sl // D
                nc.vector.tensor_add(
                    st["v"][:, si, nt // D: nt // D + nh, 0:D],
                    pt[:, 0:n_sl].rearrange("p (h d) -> p h d", d=D),
                    bv_bc[:, nt:nt + n_sl].rearrange("p (h d) -> p h d", d=D))
            return u_v

        def make_proj_atom(b, si, k):
            def u_proj():
                st = states[b]
                nt, n_sl = nts[k]
                yt = st["yt"][si]
                pt = ps_pr.tile([P, 512], F32, tag="pr", name=f"pj{b}_{si}_{k}")
                for ec in range(EC):
                    nc.tensor.matmul(
                        pt[:, 0:n_sl],
                        st["attnT"][:, ec, si * P:(si + 1) * P],
                        wproj_sb[:, ec, nt:nt + n_sl],
                        start=(ec == 0), stop=(ec == EC - 1),
                    )
                nc.vector.tensor_add(yt[:, nt:nt + n_sl], pt[:, 0:n_sl],
                                     bproj_bc[:, nt:nt + n_sl])
                if k == len(nts) - 1:
                    nc.sync.dma_start(
                        y_d[b * S + si * P: b * S + (si + 1) * P, :], yt)
            return u_proj

        def make_proj_alloc(b):
            def u():
                states[b]["yt"] = [outp.tile([P, E], act_dtype, tag=f"y{si % 2}",
                                             name=f"yt{b}_{si}") for si in range(SC)]
            return u

        done = set()
        pace = {"consumed": 0, "step": 0, "total_steps": BPC * NPAIR * NQT * SC,
                "total_atoms": 36 + 43 + 18}

        def pull_one(filler):
            key, fn = filler.popleft()
            fn()
            pace["consumed"] += 1
            if key is not None:
                done.add(key)

        def ensure(filler, keys):
            """Force-pull filler atoms (in order) until all keys are emitted —
            a consumer must never be emitted before its producers."""
            for key in keys:
                while key not in done:
                    pull_one(filler)

        def rate_pull(filler):
            """Pull filler atoms up to a uniform global schedule so the queue
            is neither front-loaded nor dry at the end of the last batch."""
            target = pace["total_atoms"] * pace["step"] // pace["total_steps"]
            while pace["consumed"] < target and filler:
                pull_one(filler)

        def run_head_pair(b, pr, qi, filler):
            """Emit one (pair, qi) attention stream: per-kc scores -> exp -> PV,
            interleaving paced filler atoms."""
            ensure(filler, [("alloc", b), ("xt", b, 0), ("xt", b, 1),
                            ("qk", b, pr, qi)])
            st = states[b]
            qkT, v, attnT, den = st["qkT"], st["v"], st["attnT"], st["den"]
            vnt = 0 if 2 * pr * D < nts[0][1] else 1
            ep = expp.tile([P, SC, HPC, QT], act_dtype, tag="exp")
            pvs = [ps_pv.tile([P, 512], F32, tag="pv", name=f"pv{b}_{pr}_{qi}_{j}")
                   for j in range(HPC)]
            for kc in range(SC):
                pace["step"] += 1
                ensure(filler, [("qk", b, EC + pr, kc * P // QT)])
                ps = ps_sc.tile([P, HPC, 512], F32, tag="sc")
                for j in range(HPC):
                    po = D * j
                    mm = nc.tensor.matmul(
                        ps[:, j, 0:QT],
                        qkT[po:po + D, EC + pr, kc * P:(kc + 1) * P],
                        qkT[po:po + D, pr, qi * QT:(qi + 1) * QT],
                        start=True, stop=True,
                        tile_position=(po, 0),
                    )
                    if FOLLOW and b == 1 and pr == 0 and qi == 0 and kc == 0:
                        tile.tile_follow(mm, log_all_deps=True)
                nc.scalar.activation(
                    ep[:, kc, :, :], ps[:, :, 0:QT], AF.Exp, scale=SCALE)
                ensure(filler, [("v", b, kc, vnt)])
                for j in range(HPC):
                    h = pr * HPC + j
                    nc.tensor.matmul(
                        pvs[j][0:DV, 0:QT],
                        v[:, kc, h, :],
                        ep[:, kc, j, :],
                        start=(kc == 0), stop=(kc == SC - 1),
                    )
                rate_pull(filler)
            for j in range(HPC):
                h = pr * HPC + j
                po = D * j
                dr = 32 * (h % 4)
                nc.vector.tensor_copy(
                    den[dr:dr + 1, h // 4, qi * QT:(qi + 1) * QT],
                    pvs[j][D:DV, 0:QT])
                nc.vector.tensor_copy(
                    attnT[po:po + D, pr, qi * QT:(qi + 1) * QT],
                    pvs[j][0:D, 0:QT])

        def make_norm_units(b, pr_lo, pr_hi):
            """Returns (u_prep, u_mul): prep gathers denominators, inverts
            them and launches the broadcast DMAs; mul applies them."""
            rbs = {}

            def u_prep():
                st = states[b]
                den = st["den"]
                h_lo, h_hi = pr_lo * HPC, pr_hi * HPC
                nh = h_hi - h_lo
                den_dense = denp.tile([H, S], F32, tag="dend_sb",
                                      name=f"dd{b}_{pr_lo}")
                for k, h in enumerate(range(h_lo, h_hi)):
                    nc.gpsimd.dma_start(den_dense[k:k + 1, :],
                                        den[32 * (h % 4):32 * (h % 4) + 1, h // 4, :])
                nc.vector.reciprocal_approx_fast(den_dense[0:nh, :],
                                                 den_dense[0:nh, :])
                den_dr = dramp.tile([H, S], act_dtype, tag="dend", name=f"dr{b}_{pr_lo}")
                nc.gpsimd.dma_start(den_dr[0:nh, :], den_dense[0:nh, :])
                for pr in range(pr_lo, pr_hi):
                    rb = rbp.tile([P, S], act_dtype, tag="rb")
                    for j in range(HPC):
                        k = (pr - pr_lo) * HPC + j
                        nc.gpsimd.dma_start(rb[D * j:D * (j + 1), :],
                                            bcast_part(den_dr[k, :], D))
                    rbs[pr] = rb

            def u_mul():
                attnT = states[b]["attnT"]
                for pr in range(pr_lo, pr_hi):
                    nc.vector.tensor_mul(attnT[:, pr, :], attnT[:, pr, :], rbs[pr])
            return u_prep, u_mul

        # ---------- emission schedule ----------
        # startup: batch-0 alloc, xT DMAs, first weight half, then prime pair 0
        make_alloc_unit(0)()
        done.add(("alloc", 0))
        make_xt_unit(0, 0)()
        done.add(("xt", 0, 0))
        emit_w_half(0)
        make_xt_unit(0, 1)()
        done.add(("xt", 0, 1))
        for m in M_ORDER[:2]:               # k(pair0), then q(pair0)
            for qi in range(NQT):
                make_qk_atom(0, m, qi)()
                done.add(("qk", 0, m, qi))
        emit_w_half(1)
        emit_wproj()

        # leftover prep-0 atoms, prioritized: pair1 qk, all v(nt0), then rest
        prep0 = deque()
        for m in M_ORDER[2:4]:
            for qi in range(NQT):
                prep0.append((("qk", 0, m, qi), make_qk_atom(0, m, qi)))
        for si in range(SC):
            prep0.append((("v", 0, si, 0), make_v_atom(0, si, 0)))
        for m in M_ORDER[4:]:
            for qi in range(NQT):
                prep0.append((("qk", 0, m, qi), make_qk_atom(0, m, qi)))
        for si in range(SC):
            prep0.append((("v", 0, si, 1), make_v_atom(0, si, 1)))

        filler = prep0
        norm_muls = {}
        for b in range(BPC):
            last = b == BPC - 1
            # build next batch's prep atoms / previous batch's proj atoms
            nxt = deque()
            if not last:
                bn = b + 1
                nxt.append((("alloc", bn), make_alloc_unit(bn)))
                nxt.append((("xt", bn, 0), make_xt_unit(bn, 0)))
                nxt.append((("xt", bn, 1), make_xt_unit(bn, 1)))
                for m in M_ORDER:
                    for qi in range(NQT):
                        nxt.append((("qk", bn, m, qi), make_qk_atom(bn, m, qi)))
                    if m == EC:          # after k(pair0), interleave v atoms
                        for si in range(SC):
                            nxt.append((("v", bn, si, 0), make_v_atom(bn, si, 0)))
                for si in range(SC):
                    nxt.append((("v", bn, si, 1), make_v_atom(bn, si, 1)))
            if b > 0:
                bp = b - 1
                nxt.append((None, norm_muls[bp]))
                nxt.append((None, make_proj_alloc(bp)))
                for si in range(SC):
                    for k in range(len(nts)):
                        nxt.append((None, make_proj_atom(bp, si, k)))
            filler.extend(nxt)

            # for the last batch, norm prep/mul per pair-group so only a tiny
            # den->recip->broadcast chain remains on the critical tail
            norm_list = []
            for pr in range(NPAIR):
                for qi in range(NQT):
                    run_head_pair(b, pr, qi, filler)
                if last:
                    if pr == 2:
                        up, um = make_norm_units(b, 0, 3)
                        up()
                        norm_list.append(um)
                    elif pr == 3:
                        norm_list[0]()
                    elif pr == 4:
                        up, um = make_norm_units(b, 3, 5)
                        up()
                        norm_list.append(um)
                    elif pr == NPAIR - 1:
                        while filler:
                            pull_one(filler)
                        up, um = make_norm_units(b, 5, NPAIR)
                        up()
                        norm_list.append(um)
                        norm_list[1]()
                        norm_list[2]()
            if not last:
                up, um = make_norm_units(b, 0, NPAIR)
                up()
                norm_muls[b] = um

        # tail: proj of the last batch
        bl = BPC - 1
        make_proj_alloc(bl)()
        for si in range(SC):
            for k in range(len(nts)):
                make_proj_atom(bl, si, k)()

        if debug_dump:
            st = states[bl]
            dq = nc.dram_tensor("dbg_qkT", [P, 2 * EC, S], act_dtype,
                                kind="ExternalOutput")
            dv = nc.dram_tensor("dbg_v", [P, SC, H, DV], act_dtype,
                                kind="ExternalOutput")
            da = nc.dram_tensor("dbg_attnT", [P, EC, S], act_dtype,
                                kind="ExternalOutput")
            dd = nc.dram_tensor("dbg_den", [P, NPLANE, S], F32,
                                kind="ExternalOutput")
            nc.sync.dma_start(dq.ap(), st["qkT"][:, :, :])
            nc.sync.dma_start(dv.ap(), st["v"][:, :, :, :])
            nc.sync.dma_start(da.ap(), st["attnT"][:, :, :])
            nc.sync.dma_start(dd.ap(), st["den"][:, :, :])

    nc.compile()
    return nc


_NC_CACHE = {}


def _get_nc():
    if "nc" not in _NC_CACHE:
        _NC_CACHE["nc"] = build_nc()
    return _NC_CACHE["nc"]


B, GS, E_FULL = 16, 1024, 768
N_CORES = 8
BPC_FULL = B // N_CORES


def make_in_maps(x, w_qkv, b_qkv, w_proj, b_proj):
    import ml_dtypes
    bf = ml_dtypes.bfloat16
    x = np.asarray(x, dtype=np.float32).astype(bf)  # [B, GS, E]
    w_qkv = np.ascontiguousarray(np.asarray(w_qkv, dtype=np.float32).astype(bf))
    b_qkv = np.ascontiguousarray(np.asarray(b_qkv, dtype=np.float32))
    w_proj = np.ascontiguousarray(np.asarray(w_proj, dtype=np.float32).astype(bf))
    b_proj = np.ascontiguousarray(np.asarray(b_proj, dtype=np.float32))
    in_maps = []
    for i in range(N_CORES):
        in_maps.append({
            "x_local": np.ascontiguousarray(
                x[i * BPC_FULL:(i + 1) * BPC_FULL].reshape(BPC_FULL * GS, E_FULL).T),
            "w_qkv": w_qkv, "b_qkv": b_qkv,
            "w_proj": w_proj, "b_proj": b_proj,
        })
    return in_maps


def gather_out(results):
    return np.concatenate(
        [r["y_local"].reshape(BPC_FULL, GS, E_FULL) for r in results],
        axis=0).astype(np.float32)


def kernel(x, w_qkv, b_qkv, w_proj, b_proj):
    from concourse.bass_utils import run_bass_kernel_spmd

    nc = _get_nc()
    in_maps = make_in_maps(x, w_qkv, b_qkv, w_proj, b_proj)
    res = run_bass_kernel_spmd(nc, in_maps, core_ids=list(range(N_CORES)))
    return gather_out(res.results)
